# revision 13
# baseline (speedup 1.0000x reference)
"""Trainium2 Bass kernel for DeterministicPhysicalLikelihoodBuilder.

Strategy (pure data-parallel over batch, 2 batches/core on 8 cores):
  - Stream [128t, 1025f] tiles of phase/comb; compute the weighted trough
    spectrum elementwise (ACT/DVE), with the row-sum fused into the final
    scalar_tensor_tensor op.
  - The einsums against the [D,F] basis (full-range + 4 subbands) are all
    partial sums of ONE matmul split at the subband boundaries along the
    contraction axis: PE-transpose trough segments to [f,t] layout, then
    accumulate per-band PSUM tiles with K-sliced matmuls. Segments are cut
    at band edges so every matmul operand starts at partition 0.
  - Normalization by mean(trough) is linear, so it is deferred to the
    channel writes (per-partition scale).
  - Channels are assembled strided into a [128, 640] tile so both outputs
    stream out as fully contiguous DMA. Per-t scalar channels (obs/rel/
    is_sound/rho and the logits weight) are computed once per batch in
    [128, 16]-wide ops to amortize instruction overhead.
"""

import os
from contextlib import ExitStack

import numpy as np

B, T, F, D = 16, 2048, 1025, 64
S = 4
NCORES = 8
BPC = B // NCORES          # batches per core
P = 128
NT = T // P                # 16 tiles of 128 rows per batch
EPS = 1e-6
NCH = 10
SOUND_SPEED = 343.0

_PROG_CACHE = {}
LAST_RESULTS = None        # stashed BassKernelResults for test harness


def _band_cuts(freq):
    """Subband boundaries as f-indices [0, c1, c2, c3, F] (bands contiguous)."""
    edges = [float(freq.min()), 500.0, 2000.0, 8000.0, float(freq.max()) + 1.0]
    cuts = [0]
    for lo, hi in zip(edges[:-1], edges[1:]):
        idx = np.nonzero((freq >= lo) & (freq < hi))[0]
        assert idx.size > 0 and int(idx[0]) == cuts[-1] and np.all(np.diff(idx) == 1)
        cuts.append(int(idx[-1]) + 1)
    assert cuts[-1] == F
    return cuts


def _segments(cuts):
    """Contraction segments (src_lo, src_hi, band), each <=128 wide, cut at
    band boundaries so every matmul K-slice starts at partition 0."""
    segs = []
    for s in range(4):
        lo, hi = cuts[s], cuts[s + 1]
        a = lo
        while a < hi:
            b = min(a + P, hi)
            segs.append((a, b, s))
            a = b
    return segs


def _build_program(cuts, denom):
    import concourse.bacc as bacc
    import concourse.tile as tile
    from concourse import masks, mybir

    dt = mybir.dt
    f32 = dt.float32
    AF = mybir.ActivationFunctionType
    ALU = mybir.AluOpType
    AX = mybir.AxisListType

    segs = _segments(cuts)
    NSEG = len(segs)
    first_seg = {}
    last_seg = {}
    for g, (_, _, s) in enumerate(segs):
        first_seg.setdefault(s, g)
        last_seg[s] = g
    # transpose groups: 4 segments per PSUM bank
    groups = [list(range(a, min(a + 4, NSEG))) for a in range(0, NSEG, 4)]

    nc = bacc.Bacc(
        "TRN2",
        target_bir_lowering=False,
        debug=False,
        enable_asserts=False,
        num_devices=NCORES,
    )

    ph_d = nc.dram_tensor("phase", [BPC, 1, T, F], f32, kind="ExternalInput").ap()
    cb_d = nc.dram_tensor("comb", [BPC, 2, T, F], f32, kind="ExternalInput").ap()
    sc_d = nc.dram_tensor("scalar", [BPC, T, S], f32, kind="ExternalInput").ap()
    ob_d = nc.dram_tensor("obs", [BPC, T, S], f32, kind="ExternalInput").ap()
    re_d = nc.dram_tensor("rel", [BPC, T, S], f32, kind="ExternalInput").ap()
    st_d = nc.dram_tensor("stpacc", [BPC, 1, T, D], f32, kind="ExternalInput").ap()
    bs_d = nc.dram_tensor("basisc", [P, NSEG * D], f32, kind="ExternalInput").ap()
    cn_d = nc.dram_tensor("consts", [P, S], f32, kind="ExternalInput").ap()
    lik_d = nc.dram_tensor("lik", [BPC, T, D, NCH], f32, kind="ExternalOutput").ap()
    lg_d = nc.dram_tensor("logits", [BPC, T, D], f32, kind="ExternalOutput").ap()

    with tile.TileContext(nc) as tc, ExitStack() as ctx:
        const_pool = ctx.enter_context(tc.tile_pool(name="const", bufs=1))
        ident = const_pool.tile([P, P], f32, name="ident")
        masks.make_identity(nc, ident[:])
        basis_sb = const_pool.tile([P, NSEG * D], f32, name="basis_sb")
        nc.sync.dma_start(basis_sb[:], bs_d)
        dinv4 = const_pool.tile([P, S], f32, name="dinv4")
        nc.sync.dma_start(dinv4[:], cn_d)

        inp = ctx.enter_context(tc.tile_pool(name="inp", bufs=3))
        work = ctx.enter_context(tc.tile_pool(name="work", bufs=2))
        small = ctx.enter_context(tc.tile_pool(name="small", bufs=3))
        batchp = ctx.enter_context(tc.tile_pool(name="batchp", bufs=2))
        outp = ctx.enter_context(tc.tile_pool(name="outp", bufs=3))
        tps = ctx.enter_context(tc.tile_pool(name="tps", bufs=3, space="PSUM"))
        bps = ctx.enter_context(tc.tile_pool(name="bps", bufs=2, space="PSUM"))

        for b in range(BPC):
            # ---- per-batch scalar channels: q4b[:, i, :] = [obs_mean,
            # rel_mean, is_sound, rho] for tile i; w_b = logits weight ----
            sct_b = batchp.tile([P, NT * S], f32, tag="sct_b")
            obs_b = batchp.tile([P, NT * S], f32, tag="obs_b")
            rel_b = batchp.tile([P, NT * S], f32, tag="rel_b")
            for i in range(NT):
                tsl = slice(i * P, (i + 1) * P)
                csl = slice(i * S, (i + 1) * S)
                nc.sync.dma_start(sct_b[:, csl], sc_d[b, tsl, :])
                nc.sync.dma_start(obs_b[:, csl], ob_d[b, tsl, :])
                nc.sync.dma_start(rel_b[:, csl], re_d[b, tsl, :])
            q4b = batchp.tile([P, NT * S], f32, tag="q4b")
            q4v = q4b[:].rearrange("p (n c) -> p n c", c=S)
            ob_v = obs_b[:].rearrange("p (n s) -> p n s", s=S)
            re_v = rel_b[:].rearrange("p (n s) -> p n s", s=S)
            sc_v = sct_b[:].rearrange("p (n s) -> p n s", s=S)
            osum_b = batchp.tile([P, NT], f32, tag="osum_b")
            nc.vector.tensor_reduce(osum_b[:], ob_v, AX.X, ALU.add)
            nc.vector.tensor_scalar_mul(q4v[:, :, 0], osum_b[:], 1.0 / S)
            rsum_b = batchp.tile([P, NT], f32, tag="rsum_b")
            nc.vector.tensor_reduce(rsum_b[:], re_v, AX.X, ALU.add)
            nc.vector.tensor_scalar_mul(q4v[:, :, 1], rsum_b[:], 1.0 / S)
            nc.vector.tensor_scalar(q4v[:, :, 2], sc_v[:, :, 0], 0.0, 1.0,
                                    op0=ALU.max, op1=ALU.min)
            ab_b = batchp.tile([P, NT], f32, tag="ab_b")
            nc.scalar.activation(ab_b[:], sc_v[:, :, 1], AF.Abs)
            nc.vector.tensor_scalar_min(q4v[:, :, 3], ab_b[:], 1.0)
            w_b = batchp.tile([P, NT], f32, tag="w_b")
            nc.vector.tensor_scalar(w_b[:], q4v[:, :, 2], 0.5 / NCH, 0.5 / NCH,
                                    op0=ALU.mult, op1=ALU.add)

            for i in range(NT):
                tsl = slice(i * P, (i + 1) * P)

                ph = inp.tile([P, F], f32, tag="ph")
                nc.sync.dma_start(ph[:], ph_d[b, 0, tsl, :])
                c0 = inp.tile([P, F], f32, tag="c0")
                nc.sync.dma_start(c0[:], cb_d[b, 0, tsl, :])
                c1 = inp.tile([P, F], f32, tag="c1")
                nc.sync.dma_start(c1[:], cb_d[b, 1, tsl, :])
                stp = small.tile([P, D], f32, tag="stp")
                nc.sync.dma_start(stp[:], st_d[b, 0, tsl, :])

                # ---- trough spectrum (elementwise, [t, f] layout) ----
                msum = small.tile([P, 1], f32, tag="msum")
                nc.vector.tensor_reduce(msum[:], ph[:], AX.X, ALU.add)
                mrow = small.tile([P, 1], f32, tag="mrow")
                nc.vector.tensor_scalar_mul(mrow[:], msum[:], 1.0 / F)
                # trough = relu(mean - x)
                trough = work.tile([P, F], f32, tag="trough")
                nc.scalar.activation(trough[:], ph[:], AF.Relu, bias=mrow[:], scale=-1.0)
                a0 = work.tile([P, F], f32, tag="a0")
                nc.scalar.activation(a0[:], c0[:], AF.Abs, scale=0.25)
                a1 = work.tile([P, F], f32, tag="a1")
                nc.scalar.activation(a1[:], c1[:], AF.Abs)
                s_t = work.tile([P, F], f32, tag="s_t")
                nc.vector.tensor_add(s_t[:], a0[:], a1[:])
                # t2 = trough * (1 + |c1| + 0.25|c0|) with fused row-sum.
                # Padded so every transpose below can read a full 128 cols.
                FT = segs[-1][0] + P
                t2 = work.tile([P, FT], f32, tag="t2")
                t2row = small.tile([P, 1], f32, tag="t2row")
                nc.vector.scalar_tensor_tensor(
                    t2[:, :F], s_t[:], 1.0, trough[:],
                    op0=ALU.add, op1=ALU.mult, accum_out=t2row[:],
                )
                if FT > F:
                    nc.gpsimd.memset(t2[:, F:FT], 0.0)

                # ---- PE transpose segments to [f, t]; grouped PSUM copies --
                # Each transpose reads 128 cols starting at the segment's lo
                # (trailing cols belong to later segments or the zero pad), so
                # all 128 PSUM rows are written; matmuls read only [0, K).
                ttr = work.tile([P, NSEG * P], f32, tag="ttr")
                for gi, grp in enumerate(groups):
                    pt = tps.tile([P, len(grp) * P], f32, tag="pt")
                    for j, g in enumerate(grp):
                        lo, _, _ = segs[g]
                        nc.tensor.transpose(
                            pt[:, j * P:(j + 1) * P],
                            t2[:, lo:lo + P], ident[:])
                    dst = ttr[:, grp[0] * P:(grp[-1] + 1) * P]
                    if gi == len(groups) - 1:
                        nc.vector.tensor_copy(dst, pt[:])
                    else:
                        nc.scalar.copy(dst, pt[:])

                # ---- band-partial matmuls (accumulate over K slices) ----
                pband = bps.tile([P, 4 * D], f32, tag="pband")
                for g, (lo, hi, s) in enumerate(segs):
                    k = hi - lo
                    nc.tensor.matmul(
                        pband[:, s * D:(s + 1) * D],
                        ttr[0:k, g * P:(g + 1) * P],
                        basis_sb[0:k, g * D:(g + 1) * D],
                        start=(g == first_seg[s]),
                        stop=(g == last_seg[s]),
                    )

                # ---- normalization scalars ----
                # rcF = 1 / max(rowsum, F*EPS) = (1/F) / max(mean, EPS)
                mx2 = small.tile([P, 1], f32, tag="mx2")
                nc.vector.tensor_scalar_max(mx2[:], t2row[:], F * EPS)
                rcF = small.tile([P, 1], f32, tag="rcF")
                nc.vector.reciprocal(rcF[:], mx2[:])
                rcd4 = small.tile([P, S], f32, tag="rcd4")
                nc.vector.tensor_scalar_mul(rcd4[:], dinv4[:], rcF[:])

                # ---- assemble likelihood channels L[t, d*10+c] ----
                L = outp.tile([P, D * NCH], f32, tag="L")
                Ldc = L[:].rearrange("p (d c) -> p d c", c=NCH)
                Lcd = L[:].rearrange("p (d c) -> p c d", c=NCH)
                pb_sd = pband[:].rearrange("p (s d) -> p s d", s=4)
                pb_ds = pband[:].rearrange("p (s d) -> p d s", s=4)
                # ch1..4 = pband * (rc/denom_s), one strided op
                rcd4_b = rcd4[:].unsqueeze(2).broadcast_to((P, 4, D))
                nc.vector.tensor_mul(Lcd[:, 1:5, :], pb_sd, rcd4_b)
                # ch0 = sum_s pband_s * rc / F
                q3 = small.tile([P, D], f32, tag="q3")
                nc.vector.tensor_reduce(q3[:], pb_ds, AX.X, ALU.add)
                nc.vector.tensor_scalar_mul(Ldc[:, :, 0], q3[:], rcF[:])
                # ch5: stp normalized
                stp_r = small.tile([P, D], f32, tag="stp_r")
                ssum = small.tile([P, 1], f32, tag="ssum")
                nc.scalar.activation(stp_r[:], stp[:], AF.Relu, accum_out=ssum[:])
                smx = small.tile([P, 1], f32, tag="smx")
                nc.vector.tensor_scalar(smx[:], ssum[:], 1.0 / D, EPS,
                                        op0=ALU.mult, op1=ALU.max)
                src = small.tile([P, 1], f32, tag="src")
                nc.vector.reciprocal(src[:], smx[:])
                nc.vector.tensor_scalar_mul(Ldc[:, :, 5], stp_r[:], src[:])
                # ch6..9: per-batch scalars broadcast across d, one op
                q4i = q4v[:, i, :].unsqueeze(1).broadcast_to((P, D, S))
                nc.scalar.activation(Ldc[:, :, 6:10], q4i, AF.Copy)

                # ---- logits = mean(channels) * (0.5 + 0.5 * is_sound) ----
                graw = small.tile([P, D], f32, tag="graw")
                nc.vector.tensor_reduce(graw[:], Ldc[:, :, :], AX.X, ALU.add)
                G = outp.tile([P, D], f32, tag="G")
                nc.vector.tensor_scalar_mul(G[:], graw[:], w_b[:, i:i + 1])

                nc.sync.dma_start(lik_d[b, tsl, :, :], Ldc[:, :, :])
                nc.sync.dma_start(lg_d[b, tsl, :], G[:])

    nc.compile()
    return nc


def _get_program(cuts, denom):
    key = tuple(cuts)
    if key not in _PROG_CACHE:
        _PROG_CACHE[key] = _build_program(cuts, denom)
    return _PROG_CACHE[key]


def _host_basis(freq, spacing):
    pattern = 0.5 * (1.0 + np.cos(
        np.float32(2.0 * np.pi) * (freq[None, :] / np.maximum(spacing[:, None], np.float32(1e-6)))
    ).astype(np.float32))
    basis = pattern / np.maximum(pattern.mean(axis=-1, keepdims=True), np.float32(EPS))
    return basis.astype(np.float32)   # [D, F]


def _host_basisc(freq, spacing, cuts):
    """Basis repacked per contraction segment: [P, NSEG*D]; segment g's rows
    live at partitions [0, K_g)."""
    basis = _host_basis(freq, spacing)       # [D, F]
    segs = _segments(cuts)
    bc = np.zeros((P, len(segs) * D), np.float32)
    for g, (lo, hi, _) in enumerate(segs):
        bc[0:hi - lo, g * D:(g + 1) * D] = basis.T[lo:hi]
    return np.ascontiguousarray(bc)


def kernel(phase, comb, scalar, scalar_observed_mask, scalar_reliable_mask,
           stpacc, frequencies_hz, spacing_grid_hz):
    global LAST_RESULTS
    from concourse.bass_utils import run_bass_kernel_spmd

    phase = np.asarray(phase, dtype=np.float32)
    comb = np.asarray(comb, dtype=np.float32)
    scalar = np.asarray(scalar, dtype=np.float32)
    obs = np.asarray(scalar_observed_mask, dtype=np.float32)
    rel = np.asarray(scalar_reliable_mask, dtype=np.float32)
    stpacc = np.asarray(stpacc, dtype=np.float32)
    freq = np.asarray(frequencies_hz, dtype=np.float32)
    spacing = np.asarray(spacing_grid_hz, dtype=np.float32)

    cuts = _band_cuts(freq)
    denom = [float(max(cuts[s + 1] - cuts[s], 1)) for s in range(4)]
    nc = _get_program(cuts, denom)

    basisc = _host_basisc(freq, spacing, cuts)
    consts = np.zeros((P, S), np.float32)
    for s in range(4):
        consts[:, s] = float(F) / denom[s]

    in_maps = []
    for c in range(NCORES):
        bsl = slice(c * BPC, (c + 1) * BPC)
        in_maps.append({
            "phase": np.ascontiguousarray(phase[bsl]),
            "comb": np.ascontiguousarray(comb[bsl]),
            "scalar": np.ascontiguousarray(scalar[bsl]),
            "obs": np.ascontiguousarray(obs[bsl]),
            "rel": np.ascontiguousarray(rel[bsl]),
            "stpacc": np.ascontiguousarray(stpacc[bsl]),
            "basisc": basisc,
            "consts": consts,
        })

    trace = bool(int(os.environ.get("BASS_KERNEL_TRACE", "0")))
    res = run_bass_kernel_spmd(nc, in_maps, list(range(NCORES)), trace=trace)
    LAST_RESULTS = res

    lik = np.concatenate([res.results[c]["lik"] for c in range(NCORES)], axis=0)
    logits = np.concatenate([res.results[c]["logits"] for c in range(NCORES)], axis=0)

    dist = (100.0 * SOUND_SPEED) / (2.0 * np.maximum(spacing, np.float32(1e-6)))
    return (lik.astype(np.float32), logits.astype(np.float32),
            spacing.astype(np.float32), dist.astype(np.float32))
